# revision 7
# baseline (speedup 1.0000x reference)
"""HGT graph update kernel for 8 Trainium2 NeuronCores.

Strategy:
  * Host folds the per-relation projections into node-level weights:
      kt_s = x @ (Wk @ blockdiag(Watt_s)) * prior_s/sqrt(C)   (per head col-block)
      mt_s = x @ (Wm @ blockdiag(Wmsg_s))
    so each edge only needs gathers:  score = <kt_s[src], q[dst]>_per-head,
    msg = mt_s[src].
  * Softmax without the max-subtraction pass (scores are O(1) here; the
    shifted/unshifted softmax are algebraically identical, fp32-safe).
  * All 2E edges are sorted by destination on the host; the 8 cores own
    contiguous 12500-node ranges, so each core completes its own segment
    softmax locally - the only collective is one AllGather of the node
    tables kt/mt (q stays core-local in SBUF).
  * Edge phase: per 128-edge block, one indirect DMA gathers [kt|mt]
    (1024B/edge) from the gathered table; q[dst] is reconstructed with a
    one-hot matmul from SBUF (no DMA); scatter-add into a PSUM window of
    128 consecutive dst nodes via a one-hot matmul.
"""

import numpy as np

N, D, H, C = 100000, 128, 8, 16
LN_EPS = 1e-3
NCORES = 8
P = 128


def _host_prep(x, src0, dst0, src1, dst1, Wk, bk, Wm, bm, Wq, bq, Wa, ba,
               Watt0, Wmsg0, Watt1, Wmsg1, prior0, prior1, skip, gamma, beta):
    """Fold weights, sort edges by dst, build per-core index records."""
    f32 = np.float32
    x = np.asarray(x, f32)
    n = x.shape[0]
    npc = n // NCORES            # nodes per core
    nwin = (npc + P - 1) // P    # windows (128-node groups) per core

    def bd(w):  # [H,C,C] -> block-diagonal [D,D]
        out = np.zeros((H * C, H * C), f32)
        for h in range(H):
            out[h * C:(h + 1) * C, h * C:(h + 1) * C] = np.asarray(w[h], f32)
        return out

    scale = 1.0 / np.sqrt(f32(C))
    cs0 = np.repeat(np.asarray(prior0, f32) * scale, C)   # [D] col scale
    cs1 = np.repeat(np.asarray(prior1, f32) * scale, C)
    Wk, bk, Wm, bm = (np.asarray(a, f32) for a in (Wk, bk, Wm, bm))
    Wkt0 = (Wk @ bd(Watt0)) * cs0; bkt0 = (bk @ bd(Watt0)) * cs0
    Wkt1 = (Wk @ bd(Watt1)) * cs1; bkt1 = (bk @ bd(Watt1)) * cs1
    Wmt0 = Wm @ bd(Wmsg0); bmt0 = bm @ bd(Wmsg0)
    Wmt1 = Wm @ bd(Wmsg1); bmt1 = bm @ bd(Wmsg1)
    # T row layout per node: [kt0 | mt0 | kt1 | mt1]  -> viewed as [2n, 256]:
    # row 2s+b = [kt_b | mt_b] of node s.
    Wbig = np.concatenate([Wkt0, Wmt0, Wkt1, Wmt1], axis=1)        # [128, 512]
    bbig = np.concatenate([bkt0, bmt0, bkt1, bmt1])                # [512]

    # ---- edges: sort by dst ----
    e0 = len(np.asarray(src0)); e1 = len(np.asarray(src1))
    src = np.concatenate([np.asarray(src0), np.asarray(src1)]).astype(np.int64)
    dst = np.concatenate([np.asarray(dst0), np.asarray(dst1)]).astype(np.int64)
    eset = np.concatenate([np.zeros(e0, np.int64), np.ones(e1, np.int64)])
    order = np.argsort(dst, kind="stable")
    src, dst, eset = src[order], dst[order], eset[order]
    kmidx = (2 * src + eset).astype(np.int32)      # row into [2n, 256] table

    # per-core, per-window edge ranges
    win_edges = [[None] * nwin for _ in range(NCORES)]
    bpw = 1
    for c in range(NCORES):
        lo_n = c * npc
        for w in range(nwin):
            a = np.searchsorted(dst, lo_n + w * P, side="left")
            b_ = np.searchsorted(dst, min(lo_n + (w + 1) * P, lo_n + npc),
                                 side="left")
            win_edges[c][w] = (a, b_)
            bpw = max(bpw, (b_ - a + P - 1) // P)

    # records: wrec[c][w] = [P, 2*bpw] int32 (col 2b: kmidx, col 2b+1:
    # rowlocal as f32 bits); rowrow[c][w] = [bpw*P] f32 (block-major)
    wrec = np.zeros((NCORES, nwin, P, 2 * bpw), np.int32)
    rowrow = np.full((NCORES, nwin, bpw * P), 1e9, f32)
    DUMMY_ROW = f32(1e9)
    for c in range(NCORES):
        lo_n = c * npc
        for w in range(nwin):
            a, b_ = win_edges[c][w]
            cnt = b_ - a
            km = np.zeros(bpw * P, np.int32)
            rl = np.full(bpw * P, DUMMY_ROW, f32)
            km[:cnt] = kmidx[a:b_]
            rl[:cnt] = (dst[a:b_] - (lo_n + w * P)).astype(f32)
            wrec[c, w, :, 0::2] = km.reshape(bpw, P).T
            wrec[c, w, :, 1::2] = rl.reshape(bpw, P).T.view(np.int32)
            rowrow[c, w, :] = rl

    alpha = float(1.0 / (1.0 + np.exp(-np.float64(np.asarray(skip)))))
    consts = dict(
        Wbig=Wbig,
        Wq=np.asarray(Wq, f32),
        Wa=np.asarray(Wa, f32),
        bias_big=np.tile(bbig[None, :], (P, 1)),
        bias_q=np.tile(np.asarray(bq, f32)[None, :], (P, 1)),
        ba_alpha=np.tile((np.asarray(ba, f32) * alpha)[None, :], (P, 1)),
        gamma_t=np.tile(np.asarray(gamma, f32)[None, :], (P, 1)),
        beta_t=np.tile(np.asarray(beta, f32)[None, :], (P, 1)),
    )
    in_maps = []
    for c in range(NCORES):
        m = dict(consts)
        m["x_slice"] = np.ascontiguousarray(x[c * npc:(c + 1) * npc])
        m["wrec"] = np.ascontiguousarray(wrec[c])
        m["rowrow"] = np.ascontiguousarray(rowrow[c])
        in_maps.append(m)
    return in_maps, dict(n=n, npc=npc, nwin=nwin, bpw=bpw, alpha=alpha)


def _build(meta):
    """Build the Bass program (shared by all 8 cores)."""
    import concourse.bass as bass
    import concourse.mybir as mybir
    import concourse.tile as tile
    from concourse.masks import make_identity

    f32 = mybir.dt.float32
    i32 = mybir.dt.int32
    AF = mybir.ActivationFunctionType
    OP = mybir.AluOpType
    n, npc, nwin, bpw = meta["n"], meta["npc"], meta["nwin"], meta["bpw"]
    alpha = meta["alpha"]

    import concourse.bacc as bacc
    nc = bacc.Bacc(trn_type="TRN2", num_devices=NCORES)

    x_slice = nc.dram_tensor("x_slice", [npc, D], f32, kind="ExternalInput")
    wrec = nc.dram_tensor("wrec", [nwin, P, 2 * bpw], i32, kind="ExternalInput")
    rowrow = nc.dram_tensor("rowrow", [nwin, bpw * P], f32, kind="ExternalInput")
    Wbig = nc.dram_tensor("Wbig", [D, 4 * D], f32, kind="ExternalInput")
    Wq = nc.dram_tensor("Wq", [D, D], f32, kind="ExternalInput")
    Wa = nc.dram_tensor("Wa", [D, D], f32, kind="ExternalInput")
    bias_big = nc.dram_tensor("bias_big", [P, 4 * D], f32, kind="ExternalInput")
    bias_q = nc.dram_tensor("bias_q", [P, D], f32, kind="ExternalInput")
    ba_alpha = nc.dram_tensor("ba_alpha", [P, D], f32, kind="ExternalInput")
    gamma_t = nc.dram_tensor("gamma_t", [P, D], f32, kind="ExternalInput")
    beta_t = nc.dram_tensor("beta_t", [P, D], f32, kind="ExternalInput")
    out = nc.dram_tensor("out", [npc, D], f32, kind="ExternalOutput")

    from contextlib import ExitStack
    with tile.TileContext(nc, num_cores=NCORES) as tc:
        with (
            tc.tile_pool(name="const", bufs=1) as cpool,
            tc.tile_pool(name="dram", bufs=1, space="DRAM") as dram,
        ):
            # ---- constants ----
            identity = cpool.tile([P, P], f32)
            make_identity(nc, identity[:])
            iota_free = cpool.tile([P, P], f32)
            nc.gpsimd.iota(iota_free[:], pattern=[[1, P]], channel_multiplier=0,
                           allow_small_or_imprecise_dtypes=True)
            iota_part = cpool.tile([P, P], f32)
            nc.gpsimd.iota(iota_part[:], pattern=[[0, P]], channel_multiplier=1,
                           allow_small_or_imprecise_dtypes=True)
            ones_row = cpool.tile([1, P], f32)
            nc.vector.memset(ones_row[:], 1.0)
            zero_col = cpool.tile([P, 1], f32)
            nc.vector.memset(zero_col[:], 0.0)
            eps_col = cpool.tile([P, 1], f32)
            nc.vector.memset(eps_col[:], LN_EPS)
            nc.const_aps.aps[(f32, 0.0)] = zero_col[:]
            nc.const_aps.aps[(f32, LN_EPS)] = eps_col[:]
            wbig_t = cpool.tile([D, 4 * D], f32)
            nc.sync.dma_start(wbig_t[:], Wbig[:])
            wq_t = cpool.tile([D, D], f32)
            nc.sync.dma_start(wq_t[:], Wq[:])
            wa_t = cpool.tile([D, D], f32)
            nc.sync.dma_start(wa_t[:], Wa[:])
            bb_t = cpool.tile([P, 4 * D], f32)
            nc.sync.dma_start(bb_t[:], bias_big[:])
            bq_t = cpool.tile([P, D], f32)
            nc.sync.dma_start(bq_t[:], bias_q[:])
            baa_t = cpool.tile([P, D], f32)
            nc.sync.dma_start(baa_t[:], ba_alpha[:])
            gam_t = cpool.tile([P, D], f32)
            nc.sync.dma_start(gam_t[:], gamma_t[:])
            bet_t = cpool.tile([P, D], f32)
            nc.sync.dma_start(bet_t[:], beta_t[:])

            # persistent SBUF state
            q_sbuf = cpool.tile([P, nwin * D], f32)
            nc.gpsimd.memset(q_sbuf[:], 0)
            pooled = cpool.tile([P, nwin * 136], f32)

            T_local = dram.tile([npc, 4 * D], f32)
            T_full = dram.tile([2 * n, 2 * D], f32)

            # ================= Phase A: projections =================
            stkA = ExitStack()
            apool = stkA.enter_context(tc.tile_pool(name="a_sb", bufs=3))
            apsum = stkA.enter_context(tc.tile_pool(name="a_ps", bufs=2, space="PSUM"))
            for t in range(nwin):
                nt = min(P, npc - t * P)
                xt = apool.tile([P, D], f32, tag="xt")
                if nt < P:
                    nc.vector.memset(xt[:], 0)
                nc.sync.dma_start(xt[:nt], x_slice[t * P:t * P + nt, :])
                xT_ps = apsum.tile([P, P], f32, tag="xT")
                nc.tensor.transpose(xT_ps[:], xt[:], identity[:])
                xTs = apool.tile([P, P], f32, tag="xTs")
                nc.scalar.copy(xTs[:], xT_ps[:])
                T_ps = apsum.tile([P, 4 * D], f32, tag="Tps")
                nc.tensor.matmul(T_ps[:], lhsT=xTs[:], rhs=wbig_t[:],
                                 start=True, stop=True)
                Tb = apool.tile([P, 4 * D], f32, tag="Tb")
                nc.vector.tensor_add(Tb[:], T_ps[:], bb_t[:])
                nc.sync.dma_start(T_local[t * P:t * P + nt, :], Tb[:nt])
                q_ps = apsum.tile([P, D], f32, tag="qps")
                nc.tensor.matmul(q_ps[:], lhsT=xTs[:], rhs=wq_t[:],
                                 start=True, stop=True)
                nc.vector.tensor_add(q_sbuf[:nt, t * D:(t + 1) * D],
                                     q_ps[:nt], bq_t[:nt])

            stkA.close()

            # ================= AllGather node tables =================
            nc.gpsimd.collective_compute(
                "AllGather",
                mybir.AluOpType.bypass,
                replica_groups=[list(range(NCORES))],
                ins=[T_local[:]],
                outs=[T_full[:]],
            )

            # ================= Phase B: edges =================
            stkB = ExitStack()
            bpool = stkB.enter_context(tc.tile_pool(name="b_sb", bufs=4))
            bpsum = stkB.enter_context(tc.tile_pool(name="b_ps", bufs=2, space="PSUM"))
            wpsum = stkB.enter_context(tc.tile_pool(name="win_ps", bufs=2, space="PSUM"))
            for w in range(nwin):
                wr = bpool.tile([P, 2 * bpw], i32, tag="wr")
                nc.sync.dma_start(wr[:], wrec[w, :, :])
                rr = bpool.tile([1, bpw * P], f32, tag="rr")
                nc.sync.dma_start(rr[:], rowrow[w:w + 1, :])
                win_ps = wpsum.tile([P, 136], f32, tag="win")
                for b in range(bpw):
                    ktmt = bpool.tile([P, 2 * D], f32, tag="ktmt")
                    nc.gpsimd.indirect_dma_start(
                        out=ktmt[:], out_offset=None,
                        in_=T_full[:],
                        in_offset=bass.IndirectOffsetOnAxis(
                            ap=wr[:, 2 * b:2 * b + 1], axis=0),
                    )
                    # SelT[j,e] = (j == rowlocal_e)
                    rb_ps = bpsum.tile([P, P], f32, tag="rb")
                    nc.tensor.matmul(rb_ps[:], lhsT=ones_row[:],
                                     rhs=rr[:, b * P:(b + 1) * P],
                                     start=True, stop=True)
                    selT = bpool.tile([P, P], f32, tag="selT")
                    nc.vector.tensor_tensor(selT[:], iota_part[:], rb_ps[:],
                                            op=OP.is_equal)
                    # q[dst] for each edge
                    qe_ps = bpsum.tile([P, P], f32, tag="qe")
                    nc.tensor.matmul(qe_ps[:], lhsT=selT[:],
                                     rhs=q_sbuf[:, w * D:(w + 1) * D],
                                     start=True, stop=True)
                    # Sel[e,j] = (rowlocal_e == j)
                    sel = bpool.tile([P, P], f32, tag="sel")
                    nc.vector.tensor_scalar(
                        sel[:], iota_free[:],
                        wr[:, 2 * b + 1:2 * b + 2].bitcast(f32), None,
                        op0=OP.is_equal)
                    prod = bpool.tile([P, D], f32, tag="prod")
                    nc.vector.tensor_mul(prod[:], ktmt[:, 0:D], qe_ps[:])
                    rhs = bpool.tile([P, 136], f32, tag="rhs")
                    nc.vector.tensor_reduce(
                        rhs[:, D:D + H], prod[:].rearrange("p (h c) -> p h c", c=C),
                        axis=mybir.AxisListType.X, op=OP.add)
                    nc.scalar.activation(rhs[:, D:D + H], rhs[:, D:D + H], AF.Exp)
                    nc.vector.tensor_tensor(
                        rhs[:, 0:D].rearrange("p (h c) -> p h c", c=C),
                        ktmt[:, D:2 * D].rearrange("p (h c) -> p h c", c=C),
                        rhs[:, D:D + H].rearrange("p (h o) -> p h o", o=1)
                            .to_broadcast([P, H, C]),
                        op=OP.mult)
                    nc.tensor.matmul(win_ps[:], lhsT=sel[:], rhs=rhs[:],
                                     start=(b == 0), stop=(b == bpw - 1))
                nc.scalar.copy(pooled[:, w * 136:(w + 1) * 136], win_ps[:])

            stkB.close()

            # ================= Phase C: aggregate + LN =================
            stkC = ExitStack()
            cpool2 = stkC.enter_context(tc.tile_pool(name="c_sb", bufs=3))
            cpsum = stkC.enter_context(tc.tile_pool(name="c_ps", bufs=2, space="PSUM"))
            for w in range(nwin):
                nt = min(P, npc - w * P)
                num = pooled[:, w * 136:w * 136 + D]
                den = pooled[:, w * 136 + D:w * 136 + D + H]
                denc = cpool2.tile([P, H], f32, tag="denc")
                nc.vector.tensor_scalar_max(denc[:], den, 1e-30)
                inv = cpool2.tile([P, H], f32, tag="inv")
                nc.vector.reciprocal(inv[:], denc[:])
                pn = cpool2.tile([P, D], f32, tag="pn")
                nc.vector.tensor_tensor(
                    pn[:].rearrange("p (h c) -> p h c", c=C),
                    num.rearrange("p (h c) -> p h c", c=C),
                    inv[:].rearrange("p (h o) -> p h o", o=1)
                        .to_broadcast([P, H, C]),
                    op=OP.mult)
                g = cpool2.tile([P, D], f32, tag="g")
                nc.scalar.activation(g[:], pn[:], AF.Gelu)
                gT_ps = cpsum.tile([P, P], f32, tag="gT")
                nc.tensor.transpose(gT_ps[:], g[:], identity[:])
                gTs = cpool2.tile([P, P], f32, tag="gTs")
                nc.scalar.copy(gTs[:], gT_ps[:])
                h_ps = cpsum.tile([P, D], f32, tag="hps")
                nc.tensor.matmul(h_ps[:], lhsT=gTs[:], rhs=wa_t[:],
                                 start=True, stop=True)
                xt2 = cpool2.tile([P, D], f32, tag="xt2")
                nc.sync.dma_start(xt2[:nt], x_slice[w * P:w * P + nt, :])
                o1 = cpool2.tile([P, D], f32, tag="o1")
                nc.vector.tensor_scalar_mul(o1[:], h_ps[:], alpha)
                nc.scalar.activation(xt2[:], xt2[:], AF.Copy, scale=1.0 - alpha)
                nc.vector.tensor_add(o1[:], o1[:], xt2[:])
                nc.vector.tensor_add(o1[:], o1[:], baa_t[:])
                # LayerNorm over features
                mu = cpool2.tile([P, 1], f32, tag="mu")
                nc.vector.tensor_reduce(mu[:], o1[:], axis=mybir.AxisListType.X,
                                        op=OP.add, negate=True)
                nc.vector.tensor_scalar_mul(mu[:], mu[:], 1.0 / D)
                xm = cpool2.tile([P, D], f32, tag="xm")
                nc.vector.tensor_scalar_add(xm[:], o1[:], mu[:, 0:1])
                sq = cpool2.tile([P, D], f32, tag="sq")
                var = cpool2.tile([P, 1], f32, tag="var")
                nc.scalar.activation(sq[:], xm[:], AF.Square,
                                     accum_out=var[:, 0:1])
                std = cpool2.tile([P, 1], f32, tag="std")
                nc.scalar.activation(std[:], var[:], AF.Sqrt, scale=1.0 / D,
                                     bias=LN_EPS)
                rinv = cpool2.tile([P, 1], f32, tag="rinv")
                nc.vector.reciprocal(rinv[:], std[:])
                xn = cpool2.tile([P, D], f32, tag="xn")
                nc.vector.tensor_scalar_mul(xn[:], xm[:], rinv[:, 0:1])
                ot = cpool2.tile([P, D], f32, tag="ot")
                nc.vector.tensor_mul(ot[:], xn[:], gam_t[:])
                nc.vector.tensor_add(ot[:], ot[:], bet_t[:])
                nc.sync.dma_start(out[w * P:w * P + nt, :], ot[:nt])
            stkC.close()

    nc.compile()
    return nc


_CACHE = {}


def kernel(**inputs):
    in_maps, meta = _host_prep(**inputs)
    key = (meta["n"], meta["npc"], meta["nwin"], meta["bpw"], meta["alpha"])
    if key not in _CACHE:
        _CACHE[key] = _build(meta)
    nc = _CACHE[key]
    from concourse.bass_utils import run_bass_kernel_spmd
    res = run_bass_kernel_spmd(nc, in_maps, core_ids=list(range(NCORES)))
    return np.concatenate([r["out"] for r in res.results], axis=0)


# revision 9
# speedup vs baseline: 1.7958x; 1.7958x over previous
"""HGT graph update kernel for 8 Trainium2 NeuronCores.

Strategy:
  * Host folds the per-relation projections into node-level weights:
      kt_s = x @ (Wk @ blockdiag(Watt_s)) * prior_s/sqrt(C)   (per head col-block)
      mt_s = x @ (Wm @ blockdiag(Wmsg_s))
    so each edge only needs gathers:  score = <kt_s[src], q[dst]>_per-head,
    msg = mt_s[src].
  * Softmax without the max-subtraction pass (scores are O(1) here; the
    shifted/unshifted softmax are algebraically identical, fp32-safe).
  * All 2E edges are sorted by destination on the host; the 8 cores own
    contiguous 12500-node ranges, so each core completes its own segment
    softmax locally - the only collective is one AllGather of the node
    tables kt/mt (q stays core-local in SBUF).
  * Edge phase: per 128-edge block, one indirect DMA gathers [kt|mt]
    (1024B/edge) from the gathered table; q[dst] is reconstructed with a
    one-hot matmul from SBUF (no DMA); scatter-add into a PSUM window of
    128 consecutive dst nodes via a one-hot matmul.
"""

import sys

if "/opt/trn_rl_repo" not in sys.path:
    sys.path.insert(0, "/opt/trn_rl_repo")
import numpy as np

N, D, H, C = 100000, 128, 8, 16
LN_EPS = 1e-3
NCORES = 8
P = 128


def _host_prep(x, src0, dst0, src1, dst1, Wk, bk, Wm, bm, Wq, bq, Wa, ba,
               Watt0, Wmsg0, Watt1, Wmsg1, prior0, prior1, skip, gamma, beta):
    """Fold weights, sort edges by dst, build per-core index records."""
    f32 = np.float32
    x = np.asarray(x, f32)
    n = x.shape[0]
    npc = n // NCORES            # nodes per core
    nwin = (npc + P - 1) // P    # windows (128-node groups) per core

    def bd(w):  # [H,C,C] -> block-diagonal [D,D]
        out = np.zeros((H * C, H * C), f32)
        for h in range(H):
            out[h * C:(h + 1) * C, h * C:(h + 1) * C] = np.asarray(w[h], f32)
        return out

    scale = 1.0 / np.sqrt(f32(C))
    cs0 = np.repeat(np.asarray(prior0, f32) * scale, C)   # [D] col scale
    cs1 = np.repeat(np.asarray(prior1, f32) * scale, C)
    Wk, bk, Wm, bm = (np.asarray(a, f32) for a in (Wk, bk, Wm, bm))
    Wkt0 = (Wk @ bd(Watt0)) * cs0; bkt0 = (bk @ bd(Watt0)) * cs0
    Wkt1 = (Wk @ bd(Watt1)) * cs1; bkt1 = (bk @ bd(Watt1)) * cs1
    Wmt0 = Wm @ bd(Wmsg0); bmt0 = bm @ bd(Wmsg0)
    Wmt1 = Wm @ bd(Wmsg1); bmt1 = bm @ bd(Wmsg1)
    # T row layout per node: [kt0 | mt0 | kt1 | mt1]  -> viewed as [2n, 256]:
    # row 2s+b = [kt_b | mt_b] of node s.
    Wbig = np.concatenate([Wkt0, Wmt0, Wkt1, Wmt1], axis=1)        # [128, 512]
    bbig = np.concatenate([bkt0, bmt0, bkt1, bmt1])                # [512]

    # ---- edges: sort by dst ----
    e0 = len(np.asarray(src0)); e1 = len(np.asarray(src1))
    src = np.concatenate([np.asarray(src0), np.asarray(src1)]).astype(np.int64)
    dst = np.concatenate([np.asarray(dst0), np.asarray(dst1)]).astype(np.int64)
    eset = np.concatenate([np.zeros(e0, np.int64), np.ones(e1, np.int64)])
    order = np.argsort(dst, kind="stable")
    src, dst, eset = src[order], dst[order], eset[order]
    kmidx = (2 * src + eset).astype(np.int32)      # row into [2n, 256] table

    # per-core, per-window edge ranges
    win_edges = [[None] * nwin for _ in range(NCORES)]
    bpw = 1
    for c in range(NCORES):
        lo_n = c * npc
        for w in range(nwin):
            a = np.searchsorted(dst, lo_n + w * P, side="left")
            b_ = np.searchsorted(dst, min(lo_n + (w + 1) * P, lo_n + npc),
                                 side="left")
            win_edges[c][w] = (a, b_)
            bpw = max(bpw, (b_ - a + P - 1) // P)

    # records: wrec[c][w] = [P, 2*bpw] int32 (col 2b: kmidx, col 2b+1:
    # rowlocal as f32 bits); rowrow[c][w] = [bpw*P] f32 (block-major)
    wrec = np.zeros((NCORES, nwin, P, 2 * bpw), np.int32)
    rowrow = np.full((NCORES, nwin, bpw * P), 1e9, f32)
    DUMMY_ROW = f32(1e9)
    for c in range(NCORES):
        lo_n = c * npc
        for w in range(nwin):
            a, b_ = win_edges[c][w]
            cnt = b_ - a
            km = np.zeros(bpw * P, np.int32)
            rl = np.full(bpw * P, DUMMY_ROW, f32)
            km[:cnt] = kmidx[a:b_]
            rl[:cnt] = (dst[a:b_] - (lo_n + w * P)).astype(f32)
            wrec[c, w, :, :bpw] = km.reshape(bpw, P).T
            wrec[c, w, :, bpw:] = rl.reshape(bpw, P).T.view(np.int32)
            rowrow[c, w, :] = rl

    alpha = float(1.0 / (1.0 + np.exp(-np.float64(np.asarray(skip)))))
    consts = dict(
        Wbig=Wbig,
        Wq=np.asarray(Wq, f32),
        Wa=np.asarray(Wa, f32),
        bias_big=np.tile(bbig[None, :], (P, 1)),
        bias_q=np.tile(np.asarray(bq, f32)[None, :], (P, 1)),
        ba_alpha=np.tile((np.asarray(ba, f32) * alpha)[None, :], (P, 1)),
        gamma_t=np.tile(np.asarray(gamma, f32)[None, :], (P, 1)),
        beta_t=np.tile(np.asarray(beta, f32)[None, :], (P, 1)),
    )
    in_maps = []
    for c in range(NCORES):
        m = dict(consts)
        m["x_slice"] = np.ascontiguousarray(x[c * npc:(c + 1) * npc])
        m["wrec"] = np.ascontiguousarray(wrec[c])
        m["rowrow"] = np.ascontiguousarray(rowrow[c])
        in_maps.append(m)
    return in_maps, dict(n=n, npc=npc, nwin=nwin, bpw=bpw, alpha=alpha)


def _build(meta):
    """Build the Bass program (shared by all 8 cores)."""
    import concourse.bass as bass
    import concourse.mybir as mybir
    import concourse.tile as tile
    from concourse.masks import make_identity

    f32 = mybir.dt.float32
    i32 = mybir.dt.int32
    AF = mybir.ActivationFunctionType
    OP = mybir.AluOpType
    n, npc, nwin, bpw = meta["n"], meta["npc"], meta["nwin"], meta["bpw"]
    alpha = meta["alpha"]

    import concourse.bacc as bacc
    nc = bacc.Bacc(trn_type="TRN2", num_devices=NCORES)

    x_slice = nc.dram_tensor("x_slice", [npc, D], f32, kind="ExternalInput")
    wrec = nc.dram_tensor("wrec", [nwin, P, 2 * bpw], i32, kind="ExternalInput")
    rowrow = nc.dram_tensor("rowrow", [nwin, bpw * P], f32, kind="ExternalInput")
    Wbig = nc.dram_tensor("Wbig", [D, 4 * D], f32, kind="ExternalInput")
    Wq = nc.dram_tensor("Wq", [D, D], f32, kind="ExternalInput")
    Wa = nc.dram_tensor("Wa", [D, D], f32, kind="ExternalInput")
    bias_big = nc.dram_tensor("bias_big", [P, 4 * D], f32, kind="ExternalInput")
    bias_q = nc.dram_tensor("bias_q", [P, D], f32, kind="ExternalInput")
    ba_alpha = nc.dram_tensor("ba_alpha", [P, D], f32, kind="ExternalInput")
    gamma_t = nc.dram_tensor("gamma_t", [P, D], f32, kind="ExternalInput")
    beta_t = nc.dram_tensor("beta_t", [P, D], f32, kind="ExternalInput")
    out = nc.dram_tensor("out", [npc, D], f32, kind="ExternalOutput")

    from contextlib import ExitStack
    with tile.TileContext(nc, num_cores=NCORES) as tc:
        with (
            tc.tile_pool(name="const", bufs=1) as cpool,
            tc.tile_pool(name="dram", bufs=1, space="DRAM") as dram,
        ):
            # ---- constants ----
            identity = cpool.tile([P, P], f32)
            make_identity(nc, identity[:])
            iota_free = cpool.tile([P, P], f32)
            nc.gpsimd.iota(iota_free[:], pattern=[[1, P]], channel_multiplier=0,
                           allow_small_or_imprecise_dtypes=True)
            iota_part = cpool.tile([P, P], f32)
            nc.gpsimd.iota(iota_part[:], pattern=[[0, P]], channel_multiplier=1,
                           allow_small_or_imprecise_dtypes=True)
            ones_row = cpool.tile([1, P], f32)
            nc.vector.memset(ones_row[:], 1.0)
            zero_col = cpool.tile([P, 1], f32)
            nc.vector.memset(zero_col[:], 0.0)
            eps_col = cpool.tile([P, 1], f32)
            nc.vector.memset(eps_col[:], LN_EPS)
            nc.const_aps.aps[(f32, 0.0)] = zero_col[:]
            nc.const_aps.aps[(f32, LN_EPS)] = eps_col[:]
            wbig_t = cpool.tile([D, 4 * D], f32)
            nc.sync.dma_start(wbig_t[:], Wbig[:])
            wq_t = cpool.tile([D, D], f32)
            nc.sync.dma_start(wq_t[:], Wq[:])
            wa_t = cpool.tile([D, D], f32)
            nc.sync.dma_start(wa_t[:], Wa[:])
            bb_t = cpool.tile([P, 4 * D], f32)
            nc.sync.dma_start(bb_t[:], bias_big[:])
            bq_t = cpool.tile([P, D], f32)
            nc.sync.dma_start(bq_t[:], bias_q[:])
            baa_t = cpool.tile([P, D], f32)
            nc.sync.dma_start(baa_t[:], ba_alpha[:])
            gam_t = cpool.tile([P, D], f32)
            nc.sync.dma_start(gam_t[:], gamma_t[:])
            bet_t = cpool.tile([P, D], f32)
            nc.sync.dma_start(bet_t[:], beta_t[:])

            # persistent SBUF state
            q_sbuf = cpool.tile([P, nwin * D], f32)
            nc.gpsimd.memset(q_sbuf[:], 0)
            pooled = cpool.tile([P, nwin * 136], f32)

            T_local = dram.tile([npc, 4 * D], f32)
            T_full = dram.tile([2 * n, 2 * D], f32)

            # ================= Phase A: projections =================
            stkA = ExitStack()
            apool = stkA.enter_context(tc.tile_pool(name="a_sb", bufs=3))
            apsum = stkA.enter_context(tc.tile_pool(name="a_ps", bufs=2, space="PSUM"))
            for t in range(nwin):
                nt = min(P, npc - t * P)
                xt = apool.tile([P, D], f32, tag="xt")
                if nt < P:
                    nc.vector.memset(xt[:], 0)
                nc.sync.dma_start(xt[:nt], x_slice[t * P:t * P + nt, :])
                xT_ps = apsum.tile([P, P], f32, tag="xT")
                nc.tensor.transpose(xT_ps[:], xt[:], identity[:])
                xTs = apool.tile([P, P], f32, tag="xTs")
                nc.scalar.copy(xTs[:], xT_ps[:])
                T_ps = apsum.tile([P, 4 * D], f32, tag="Tps")
                nc.tensor.matmul(T_ps[:], lhsT=xTs[:], rhs=wbig_t[:],
                                 start=True, stop=True)
                Tb = apool.tile([P, 4 * D], f32, tag="Tb")
                nc.vector.tensor_add(Tb[:], T_ps[:], bb_t[:])
                nc.sync.dma_start(T_local[t * P:t * P + nt, :], Tb[:nt])
                q_ps = apsum.tile([P, D], f32, tag="qps")
                nc.tensor.matmul(q_ps[:], lhsT=xTs[:], rhs=wq_t[:],
                                 start=True, stop=True)
                nc.vector.tensor_add(q_sbuf[:nt, t * D:(t + 1) * D],
                                     q_ps[:nt], bq_t[:nt])

            stkA.close()

            # ================= AllGather node tables =================
            nc.gpsimd.collective_compute(
                "AllGather",
                mybir.AluOpType.bypass,
                replica_groups=[list(range(NCORES))],
                ins=[T_local[:]],
                outs=[T_full[:]],
            )

            # ================= Phase B: edges =================
            stkB = ExitStack()
            bpool = stkB.enter_context(tc.tile_pool(name="b_sb", bufs=4))
            bpsum = stkB.enter_context(tc.tile_pool(name="b_ps", bufs=2, space="PSUM"))
            wpsum = stkB.enter_context(tc.tile_pool(name="win_ps", bufs=2, space="PSUM"))
            for w in range(nwin):
                wr = bpool.tile([P, 2 * bpw], i32, tag="wr")
                nc.sync.dma_start(wr[:], wrec[w, :, :])
                rr = bpool.tile([1, bpw * P], f32, tag="rr")
                nc.sync.dma_start(rr[:], rowrow[w:w + 1, :])
                win_ps = wpsum.tile([P, 136], f32, tag="win")
                for b in range(bpw):
                    ktmt = bpool.tile([P, 2 * D], f32, tag="ktmt")
                    nc.gpsimd.indirect_dma_start(
                        out=ktmt[:], out_offset=None,
                        in_=T_full[:],
                        in_offset=bass.IndirectOffsetOnAxis(
                            ap=wr[:, b:b + 1], axis=0),
                    )
                    # SelT[j,e] = (j == rowlocal_e)
                    rb_ps = bpsum.tile([P, P], f32, tag="rb")
                    nc.tensor.matmul(rb_ps[:], lhsT=ones_row[:],
                                     rhs=rr[:, b * P:(b + 1) * P],
                                     start=True, stop=True)
                    selT = bpool.tile([P, P], f32, tag="selT")
                    nc.vector.tensor_tensor(selT[:], iota_part[:], rb_ps[:],
                                            op=OP.is_equal)
                    # q[dst] for each edge
                    qe_ps = bpsum.tile([P, P], f32, tag="qe")
                    nc.tensor.matmul(qe_ps[:], lhsT=selT[:],
                                     rhs=q_sbuf[:, w * D:(w + 1) * D],
                                     start=True, stop=True)
                    # Sel[e,j] = (rowlocal_e == j)
                    sel = bpool.tile([P, P], f32, tag="sel")
                    nc.vector.tensor_scalar(
                        sel[:], iota_free[:],
                        wr[:, bpw + b:bpw + b + 1].bitcast(f32), None,
                        op0=OP.is_equal)
                    prod = bpool.tile([P, D], f32, tag="prod")
                    nc.vector.tensor_mul(prod[:], ktmt[:][:, 0:D], qe_ps[:])
                    rhs = bpool.tile([P, 136], f32, tag="rhs")
                    nc.vector.tensor_reduce(
                        rhs[:, D:D + H], prod[:].rearrange("p (h c) -> p h c", c=C),
                        axis=mybir.AxisListType.X, op=OP.add)
                    nc.scalar.activation(rhs[:, D:D + H], rhs[:, D:D + H], AF.Exp)
                    nc.vector.tensor_tensor(
                        rhs[:, 0:D].rearrange("p (h c) -> p h c", c=C),
                        ktmt[:][:, D:2 * D].rearrange("p (h c) -> p h c", c=C),
                        rhs[:, D:D + H].rearrange("p (h o) -> p h o", o=1)
                            .to_broadcast([P, H, C]),
                        op=OP.mult)
                    nc.tensor.matmul(win_ps[:], lhsT=sel[:], rhs=rhs[:],
                                     start=(b == 0), stop=(b == bpw - 1))
                nc.scalar.copy(pooled[:, w * 136:(w + 1) * 136], win_ps[:])

            stkB.close()

            # ================= Phase C: aggregate + LN =================
            stkC = ExitStack()
            cpool2 = stkC.enter_context(tc.tile_pool(name="c_sb", bufs=3))
            cpsum = stkC.enter_context(tc.tile_pool(name="c_ps", bufs=2, space="PSUM"))
            for w in range(nwin):
                nt = min(P, npc - w * P)
                num = pooled[:, w * 136:w * 136 + D]
                den = pooled[:, w * 136 + D:w * 136 + D + H]
                denc = cpool2.tile([P, H], f32, tag="denc")
                nc.vector.tensor_scalar_max(denc[:], den, 1e-30)
                inv = cpool2.tile([P, H], f32, tag="inv")
                nc.vector.reciprocal(inv[:], denc[:])
                pn = cpool2.tile([P, D], f32, tag="pn")
                nc.vector.tensor_tensor(
                    pn[:].rearrange("p (h c) -> p h c", c=C),
                    num.rearrange("p (h c) -> p h c", c=C),
                    inv[:].rearrange("p (h o) -> p h o", o=1)
                        .to_broadcast([P, H, C]),
                    op=OP.mult)
                g = cpool2.tile([P, D], f32, tag="g")
                nc.scalar.activation(g[:], pn[:], AF.Gelu)
                gT_ps = cpsum.tile([P, P], f32, tag="gT")
                nc.tensor.transpose(gT_ps[:], g[:], identity[:])
                gTs = cpool2.tile([P, P], f32, tag="gTs")
                nc.scalar.copy(gTs[:], gT_ps[:])
                h_ps = cpsum.tile([P, D], f32, tag="hps")
                nc.tensor.matmul(h_ps[:], lhsT=gTs[:], rhs=wa_t[:],
                                 start=True, stop=True)
                xt2 = cpool2.tile([P, D], f32, tag="xt2")
                nc.sync.dma_start(xt2[:nt], x_slice[w * P:w * P + nt, :])
                o1 = cpool2.tile([P, D], f32, tag="o1")
                nc.vector.tensor_scalar_mul(o1[:], h_ps[:], alpha)
                nc.scalar.activation(xt2[:], xt2[:], AF.Copy, scale=1.0 - alpha)
                nc.vector.tensor_add(o1[:], o1[:], xt2[:])
                nc.vector.tensor_add(o1[:], o1[:], baa_t[:])
                # LayerNorm over features
                mu = cpool2.tile([P, 1], f32, tag="mu")
                nc.vector.tensor_reduce(mu[:], o1[:], axis=mybir.AxisListType.X,
                                        op=OP.add, negate=True)
                nc.vector.tensor_scalar_mul(mu[:], mu[:], 1.0 / D)
                xm = cpool2.tile([P, D], f32, tag="xm")
                nc.vector.tensor_scalar_add(xm[:], o1[:], mu[:, 0:1])
                sq = cpool2.tile([P, D], f32, tag="sq")
                var = cpool2.tile([P, 1], f32, tag="var")
                nc.scalar.activation(sq[:], xm[:], AF.Square,
                                     accum_out=var[:, 0:1])
                std = cpool2.tile([P, 1], f32, tag="std")
                nc.scalar.activation(std[:], var[:], AF.Sqrt, scale=1.0 / D,
                                     bias=LN_EPS)
                rinv = cpool2.tile([P, 1], f32, tag="rinv")
                nc.vector.reciprocal(rinv[:], std[:])
                xn = cpool2.tile([P, D], f32, tag="xn")
                nc.vector.tensor_scalar_mul(xn[:], xm[:], rinv[:, 0:1])
                ot = cpool2.tile([P, D], f32, tag="ot")
                nc.vector.tensor_mul(ot[:], xn[:], gam_t[:])
                nc.vector.tensor_add(ot[:], ot[:], bet_t[:])
                nc.sync.dma_start(out[w * P:w * P + nt, :], ot[:nt])
            stkC.close()

    nc.compile()
    return nc


_CACHE = {}


def kernel(**inputs):
    in_maps, meta = _host_prep(**inputs)
    key = (meta["n"], meta["npc"], meta["nwin"], meta["bpw"], meta["alpha"])
    if key not in _CACHE:
        _CACHE[key] = _build(meta)
    nc = _CACHE[key]
    from concourse.bass_utils import run_bass_kernel_spmd
    res = run_bass_kernel_spmd(nc, in_maps, core_ids=list(range(NCORES)))
    return np.concatenate([r["out"] for r in res.results], axis=0)


# revision 10
# speedup vs baseline: 1.8936x; 1.0545x over previous
"""HGT graph update kernel for 8 Trainium2 NeuronCores.

Strategy:
  * Host folds the per-relation projections into node-level weights:
      kt_s = x @ (Wk @ blockdiag(Watt_s)) * prior_s/sqrt(C)   (per head col-block)
      mt_s = x @ (Wm @ blockdiag(Wmsg_s))
    so each edge only needs gathers:  score = <kt_s[src], q[dst]>_per-head,
    msg = mt_s[src].
  * Softmax without the max-subtraction pass (scores are O(1) here; the
    shifted/unshifted softmax are algebraically identical, fp32-safe).
  * All 2E edges are sorted by destination on the host; the 8 cores own
    contiguous 12500-node ranges, so each core completes its own segment
    softmax locally - the only collective is one AllGather of the node
    tables kt/mt (q stays core-local in SBUF).
  * Edge phase: per 128-edge block, one indirect DMA gathers [kt|mt]
    (1024B/edge) from the gathered table; q[dst] is reconstructed with a
    one-hot matmul from SBUF (no DMA); scatter-add into a PSUM window of
    128 consecutive dst nodes via a one-hot matmul.
"""

import sys

if "/opt/trn_rl_repo" not in sys.path:
    sys.path.insert(0, "/opt/trn_rl_repo")
import numpy as np

N, D, H, C = 100000, 128, 8, 16
LN_EPS = 1e-3
NCORES = 8
P = 128


def _host_prep(x, src0, dst0, src1, dst1, Wk, bk, Wm, bm, Wq, bq, Wa, ba,
               Watt0, Wmsg0, Watt1, Wmsg1, prior0, prior1, skip, gamma, beta):
    """Fold weights, sort edges by dst, build per-core index records."""
    f32 = np.float32
    x = np.asarray(x, f32)
    n = x.shape[0]
    npc = n // NCORES            # nodes per core
    nwin = (npc + P - 1) // P    # windows (128-node groups) per core

    def bd(w):  # [H,C,C] -> block-diagonal [D,D]
        out = np.zeros((H * C, H * C), f32)
        for h in range(H):
            out[h * C:(h + 1) * C, h * C:(h + 1) * C] = np.asarray(w[h], f32)
        return out

    scale = 1.0 / np.sqrt(f32(C))
    cs0 = np.repeat(np.asarray(prior0, f32) * scale, C)   # [D] col scale
    cs1 = np.repeat(np.asarray(prior1, f32) * scale, C)
    Wk, bk, Wm, bm = (np.asarray(a, f32) for a in (Wk, bk, Wm, bm))
    Wkt0 = (Wk @ bd(Watt0)) * cs0; bkt0 = (bk @ bd(Watt0)) * cs0
    Wkt1 = (Wk @ bd(Watt1)) * cs1; bkt1 = (bk @ bd(Watt1)) * cs1
    Wmt0 = Wm @ bd(Wmsg0); bmt0 = bm @ bd(Wmsg0)
    Wmt1 = Wm @ bd(Wmsg1); bmt1 = bm @ bd(Wmsg1)
    # T row layout per node: [kt0 | mt0 | kt1 | mt1]  -> viewed as [2n, 256]:
    # row 2s+b = [kt_b | mt_b] of node s.
    Wbig = np.concatenate([Wkt0, Wmt0, Wkt1, Wmt1], axis=1)        # [128, 512]
    bbig = np.concatenate([bkt0, bmt0, bkt1, bmt1])                # [512]

    # ---- edges: sort by dst ----
    e0 = len(np.asarray(src0)); e1 = len(np.asarray(src1))
    src = np.concatenate([np.asarray(src0), np.asarray(src1)]).astype(np.int64)
    dst = np.concatenate([np.asarray(dst0), np.asarray(dst1)]).astype(np.int64)
    eset = np.concatenate([np.zeros(e0, np.int64), np.ones(e1, np.int64)])
    order = np.argsort(dst, kind="stable")
    src, dst, eset = src[order], dst[order], eset[order]
    kmidx = (2 * src + eset).astype(np.int32)      # row into [2n, 256] table

    # per-core, per-window edge ranges
    win_edges = [[None] * nwin for _ in range(NCORES)]
    bpw = 1
    for c in range(NCORES):
        lo_n = c * npc
        for w in range(nwin):
            a = np.searchsorted(dst, lo_n + w * P, side="left")
            b_ = np.searchsorted(dst, min(lo_n + (w + 1) * P, lo_n + npc),
                                 side="left")
            win_edges[c][w] = (a, b_)
            bpw = max(bpw, (b_ - a + P - 1) // P)

    # records: wrec[c][w] = [P, 2*bpw] int32 (col 2b: kmidx, col 2b+1:
    # rowlocal as f32 bits); rowrow[c][w] = [bpw*P] f32 (block-major)
    wrec = np.zeros((NCORES, nwin, P, 2 * bpw), np.int32)
    rowrow = np.full((NCORES, nwin, bpw * P), 1e9, f32)
    DUMMY_ROW = f32(1e9)
    for c in range(NCORES):
        lo_n = c * npc
        for w in range(nwin):
            a, b_ = win_edges[c][w]
            cnt = b_ - a
            km = np.zeros(bpw * P, np.int32)
            rl = np.full(bpw * P, DUMMY_ROW, f32)
            km[:cnt] = kmidx[a:b_]
            rl[:cnt] = (dst[a:b_] - (lo_n + w * P)).astype(f32)
            wrec[c, w, :, :bpw] = km.reshape(bpw, P).T
            wrec[c, w, :, bpw:] = rl.reshape(bpw, P).T.view(np.int32)
            rowrow[c, w, :] = rl

    alpha = float(1.0 / (1.0 + np.exp(-np.float64(np.asarray(skip)))))
    consts = dict(
        Wbig=Wbig,
        Wq=np.asarray(Wq, f32),
        Wa=np.asarray(Wa, f32),
        bias_big=np.tile(bbig[None, :], (P, 1)),
        bias_q=np.tile(np.asarray(bq, f32)[None, :], (P, 1)),
        ba_alpha=np.tile((np.asarray(ba, f32) * alpha)[None, :], (P, 1)),
        gamma_t=np.tile(np.asarray(gamma, f32)[None, :], (P, 1)),
        beta_t=np.tile(np.asarray(beta, f32)[None, :], (P, 1)),
    )
    in_maps = []
    for c in range(NCORES):
        m = dict(consts)
        m["x_slice"] = np.ascontiguousarray(x[c * npc:(c + 1) * npc])
        m["wrec"] = np.ascontiguousarray(wrec[c])
        m["rowrow"] = np.ascontiguousarray(rowrow[c])
        in_maps.append(m)
    return in_maps, dict(n=n, npc=npc, nwin=nwin, bpw=bpw, alpha=alpha)


def _build(meta):
    """Build the Bass program (shared by all 8 cores)."""
    import concourse.bass as bass
    import concourse.mybir as mybir
    import concourse.tile as tile
    from concourse.masks import make_identity

    f32 = mybir.dt.float32
    i32 = mybir.dt.int32
    AF = mybir.ActivationFunctionType
    OP = mybir.AluOpType
    n, npc, nwin, bpw = meta["n"], meta["npc"], meta["nwin"], meta["bpw"]
    alpha = meta["alpha"]

    import concourse.bacc as bacc
    nc = bacc.Bacc(trn_type="TRN2", num_devices=NCORES)

    x_slice = nc.dram_tensor("x_slice", [npc, D], f32, kind="ExternalInput")
    wrec = nc.dram_tensor("wrec", [nwin, P, 2 * bpw], i32, kind="ExternalInput")
    rowrow = nc.dram_tensor("rowrow", [nwin, bpw * P], f32, kind="ExternalInput")
    Wbig = nc.dram_tensor("Wbig", [D, 4 * D], f32, kind="ExternalInput")
    Wq = nc.dram_tensor("Wq", [D, D], f32, kind="ExternalInput")
    Wa = nc.dram_tensor("Wa", [D, D], f32, kind="ExternalInput")
    bias_big = nc.dram_tensor("bias_big", [P, 4 * D], f32, kind="ExternalInput")
    bias_q = nc.dram_tensor("bias_q", [P, D], f32, kind="ExternalInput")
    ba_alpha = nc.dram_tensor("ba_alpha", [P, D], f32, kind="ExternalInput")
    gamma_t = nc.dram_tensor("gamma_t", [P, D], f32, kind="ExternalInput")
    beta_t = nc.dram_tensor("beta_t", [P, D], f32, kind="ExternalInput")
    out = nc.dram_tensor("out", [npc, D], f32, kind="ExternalOutput")

    from contextlib import ExitStack
    with tile.TileContext(nc, num_cores=NCORES) as tc:
        with (
            tc.tile_pool(name="const", bufs=1) as cpool,
            tc.tile_pool(name="dram", bufs=1, space="DRAM") as dram,
        ):
            # ---- constants ----
            identity = cpool.tile([P, P], f32)
            make_identity(nc, identity[:])
            iota_free = cpool.tile([P, P], f32)
            nc.gpsimd.iota(iota_free[:], pattern=[[1, P]], channel_multiplier=0,
                           allow_small_or_imprecise_dtypes=True)
            iota_part = cpool.tile([P, P], f32)
            nc.gpsimd.iota(iota_part[:], pattern=[[0, P]], channel_multiplier=1,
                           allow_small_or_imprecise_dtypes=True)
            ones_row = cpool.tile([1, P], f32)
            nc.vector.memset(ones_row[:], 1.0)
            zero_col = cpool.tile([P, 1], f32)
            nc.vector.memset(zero_col[:], 0.0)
            eps_col = cpool.tile([P, 1], f32)
            nc.vector.memset(eps_col[:], LN_EPS)
            nc.const_aps.aps[(f32, 0.0)] = zero_col[:]
            nc.const_aps.aps[(f32, LN_EPS)] = eps_col[:]
            wbig_t = cpool.tile([D, 4 * D], f32)
            nc.sync.dma_start(wbig_t[:], Wbig[:])
            wq_t = cpool.tile([D, D], f32)
            nc.sync.dma_start(wq_t[:], Wq[:])
            wa_t = cpool.tile([D, D], f32)
            nc.sync.dma_start(wa_t[:], Wa[:])
            bb_t = cpool.tile([P, 4 * D], f32)
            nc.sync.dma_start(bb_t[:], bias_big[:])
            bq_t = cpool.tile([P, D], f32)
            nc.sync.dma_start(bq_t[:], bias_q[:])
            baa_t = cpool.tile([P, D], f32)
            nc.sync.dma_start(baa_t[:], ba_alpha[:])
            gam_t = cpool.tile([P, D], f32)
            nc.sync.dma_start(gam_t[:], gamma_t[:])
            bet_t = cpool.tile([P, D], f32)
            nc.sync.dma_start(bet_t[:], beta_t[:])

            # persistent SBUF state
            q_sbuf = cpool.tile([P, nwin * D], f32)
            nc.gpsimd.memset(q_sbuf[:], 0)
            pooled = cpool.tile([P, nwin * 136], f32)

            T_local = dram.tile([npc, 4 * D], f32)
            T_full = dram.tile([2 * n, 2 * D], f32)

            # ================= Phase A: projections =================
            stkA = ExitStack()
            apool = stkA.enter_context(tc.tile_pool(name="a_sb", bufs=3))
            apsum = stkA.enter_context(tc.tile_pool(name="a_ps", bufs=2, space="PSUM"))
            for t in range(nwin):
                nt = min(P, npc - t * P)
                xt = apool.tile([P, D], f32, tag="xt")
                if nt < P:
                    nc.vector.memset(xt[:], 0)
                nc.sync.dma_start(xt[:nt], x_slice[t * P:t * P + nt, :])
                xT_ps = apsum.tile([P, P], f32, tag="xT")
                nc.tensor.transpose(xT_ps[:], xt[:], identity[:])
                xTs = apool.tile([P, P], f32, tag="xTs")
                nc.scalar.copy(xTs[:], xT_ps[:])
                T_ps = apsum.tile([P, 4 * D], f32, tag="Tps")
                nc.tensor.matmul(T_ps[:], lhsT=xTs[:], rhs=wbig_t[:],
                                 start=True, stop=True)
                Tb = apool.tile([P, 4 * D], f32, tag="Tb")
                nc.vector.tensor_add(Tb[:], T_ps[:], bb_t[:])
                nc.sync.dma_start(T_local[t * P:t * P + nt, :], Tb[:nt])
                q_ps = apsum.tile([P, D], f32, tag="qps")
                nc.tensor.matmul(q_ps[:], lhsT=xTs[:], rhs=wq_t[:],
                                 start=True, stop=True)
                nc.vector.tensor_add(q_sbuf[:nt, t * D:(t + 1) * D],
                                     q_ps[:nt], bq_t[:nt])

            stkA.close()

            # ================= AllGather node tables =================
            nc.gpsimd.collective_compute(
                "AllGather",
                mybir.AluOpType.bypass,
                replica_groups=[list(range(NCORES))],
                ins=[T_local[:]],
                outs=[T_full[:]],
            )

            # ================= Phase B: edges =================
            stkB = ExitStack()
            bpool = stkB.enter_context(tc.tile_pool(name="b_sb", bufs=4))
            bpsum = stkB.enter_context(tc.tile_pool(name="b_ps", bufs=3, space="PSUM"))
            wpsum = stkB.enter_context(tc.tile_pool(name="win_ps", bufs=2, space="PSUM"))
            for w in range(nwin):
                wr = bpool.tile([P, 2 * bpw], i32, tag="wr")
                nc.sync.dma_start(wr[:], wrec[w, :, :])
                rr = bpool.tile([1, bpw * P], f32, tag="rr")
                nc.sync.dma_start(rr[:], rowrow[w:w + 1, :])
                win_ps = wpsum.tile([P, 136], f32, tag="win")
                for b in range(bpw):
                    ktmt = bpool.tile([P, 2 * D], f32, tag="ktmt", bufs=8)
                    nc.gpsimd.indirect_dma_start(
                        out=ktmt[:], out_offset=None,
                        in_=T_full[:],
                        in_offset=bass.IndirectOffsetOnAxis(
                            ap=wr[:, b:b + 1], axis=0),
                    )
                    # SelT[j,e] = (j == rowlocal_e)
                    rb_ps = bpsum.tile([P, P], f32, tag="rb")
                    nc.tensor.matmul(rb_ps[:], lhsT=ones_row[:],
                                     rhs=rr[:, b * P:(b + 1) * P],
                                     start=True, stop=True)
                    selT = bpool.tile([P, P], f32, tag="selT")
                    nc.vector.tensor_tensor(selT[:], iota_part[:], rb_ps[:],
                                            op=OP.is_equal)
                    # q[dst] for each edge
                    qe_ps = bpsum.tile([P, P], f32, tag="qe")
                    nc.tensor.matmul(qe_ps[:], lhsT=selT[:],
                                     rhs=q_sbuf[:, w * D:(w + 1) * D],
                                     start=True, stop=True)
                    # Sel[e,j] = (rowlocal_e == j)
                    sel = bpool.tile([P, P], f32, tag="sel")
                    nc.vector.tensor_scalar(
                        sel[:], iota_free[:],
                        wr[:, bpw + b:bpw + b + 1].bitcast(f32), None,
                        op0=OP.is_equal)
                    prod = bpool.tile([P, D], f32, tag="prod")
                    nc.vector.tensor_mul(prod[:], ktmt[:][:, 0:D], qe_ps[:])
                    rhs = bpool.tile([P, 136], f32, tag="rhs")
                    nc.vector.tensor_reduce(
                        rhs[:, D:D + H], prod[:].rearrange("p (h c) -> p h c", c=C),
                        axis=mybir.AxisListType.X, op=OP.add)
                    nc.scalar.activation(rhs[:, D:D + H], rhs[:, D:D + H], AF.Exp)
                    nc.vector.tensor_tensor(
                        rhs[:, 0:D].rearrange("p (h c) -> p h c", c=C),
                        ktmt[:][:, D:2 * D].rearrange("p (h c) -> p h c", c=C),
                        rhs[:, D:D + H].rearrange("p (h o) -> p h o", o=1)
                            .to_broadcast([P, H, C]),
                        op=OP.mult)
                    nc.tensor.matmul(win_ps[:], lhsT=sel[:], rhs=rhs[:],
                                     start=(b == 0), stop=(b == bpw - 1))
                nc.scalar.copy(pooled[:, w * 136:(w + 1) * 136], win_ps[:])

            stkB.close()

            # ================= Phase C: aggregate + LN =================
            stkC = ExitStack()
            cpool2 = stkC.enter_context(tc.tile_pool(name="c_sb", bufs=3))
            cpsum = stkC.enter_context(tc.tile_pool(name="c_ps", bufs=2, space="PSUM"))
            for w in range(nwin):
                nt = min(P, npc - w * P)
                num = pooled[:, w * 136:w * 136 + D]
                den = pooled[:, w * 136 + D:w * 136 + D + H]
                denc = cpool2.tile([P, H], f32, tag="denc")
                nc.vector.tensor_scalar_max(denc[:], den, 1e-30)
                inv = cpool2.tile([P, H], f32, tag="inv")
                nc.vector.reciprocal(inv[:], denc[:])
                pn = cpool2.tile([P, D], f32, tag="pn")
                nc.vector.tensor_tensor(
                    pn[:].rearrange("p (h c) -> p h c", c=C),
                    num.rearrange("p (h c) -> p h c", c=C),
                    inv[:].rearrange("p (h o) -> p h o", o=1)
                        .to_broadcast([P, H, C]),
                    op=OP.mult)
                g = cpool2.tile([P, D], f32, tag="g")
                nc.scalar.activation(g[:], pn[:], AF.Gelu)
                gT_ps = cpsum.tile([P, P], f32, tag="gT")
                nc.tensor.transpose(gT_ps[:], g[:], identity[:])
                gTs = cpool2.tile([P, P], f32, tag="gTs")
                nc.scalar.copy(gTs[:], gT_ps[:])
                h_ps = cpsum.tile([P, D], f32, tag="hps")
                nc.tensor.matmul(h_ps[:], lhsT=gTs[:], rhs=wa_t[:],
                                 start=True, stop=True)
                xt2 = cpool2.tile([P, D], f32, tag="xt2")
                nc.sync.dma_start(xt2[:nt], x_slice[w * P:w * P + nt, :])
                o1 = cpool2.tile([P, D], f32, tag="o1")
                nc.vector.tensor_scalar_mul(o1[:], h_ps[:], alpha)
                nc.scalar.activation(xt2[:], xt2[:], AF.Copy, scale=1.0 - alpha)
                nc.vector.tensor_add(o1[:], o1[:], xt2[:])
                nc.vector.tensor_add(o1[:], o1[:], baa_t[:])
                # LayerNorm over features
                mu = cpool2.tile([P, 1], f32, tag="mu")
                nc.vector.tensor_reduce(mu[:], o1[:], axis=mybir.AxisListType.X,
                                        op=OP.add, negate=True)
                nc.vector.tensor_scalar_mul(mu[:], mu[:], 1.0 / D)
                xm = cpool2.tile([P, D], f32, tag="xm")
                nc.vector.tensor_scalar_add(xm[:], o1[:], mu[:, 0:1])
                sq = cpool2.tile([P, D], f32, tag="sq")
                var = cpool2.tile([P, 1], f32, tag="var")
                nc.scalar.activation(sq[:], xm[:], AF.Square,
                                     accum_out=var[:, 0:1])
                std = cpool2.tile([P, 1], f32, tag="std")
                nc.scalar.activation(std[:], var[:], AF.Sqrt, scale=1.0 / D,
                                     bias=LN_EPS)
                rinv = cpool2.tile([P, 1], f32, tag="rinv")
                nc.vector.reciprocal(rinv[:], std[:])
                xn = cpool2.tile([P, D], f32, tag="xn")
                nc.vector.tensor_scalar_mul(xn[:], xm[:], rinv[:, 0:1])
                ot = cpool2.tile([P, D], f32, tag="ot")
                nc.vector.tensor_mul(ot[:], xn[:], gam_t[:])
                nc.vector.tensor_add(ot[:], ot[:], bet_t[:])
                nc.sync.dma_start(out[w * P:w * P + nt, :], ot[:nt])
            stkC.close()

    nc.compile()
    return nc


_CACHE = {}


def kernel(**inputs):
    in_maps, meta = _host_prep(**inputs)
    key = (meta["n"], meta["npc"], meta["nwin"], meta["bpw"], meta["alpha"])
    if key not in _CACHE:
        _CACHE[key] = _build(meta)
    nc = _CACHE[key]
    from concourse.bass_utils import run_bass_kernel_spmd
    res = run_bass_kernel_spmd(nc, in_maps, core_ids=list(range(NCORES)))
    return np.concatenate([r["out"] for r in res.results], axis=0)
